# revision 30
# baseline (speedup 1.0000x reference)
"""Trainium2 Bass kernel for nn_CNNcond_9723805958518 (dense_cnn).

Computation (see reference.py): for embedded [B,S,D], filt [K*D,1], bias [1]:
    out[b, i] = sum_{k<K, d<D} embedded[b, i+k, d] * w[k, d] + bias
with K-1 zero frames padded past the end of the sequence
(B=32, S=4096, D=512, K=16).

Distribution: pure data parallelism over batch - 8 NeuronCores x 4 batches,
no collectives; each core gets its x slice pre-transposed and chunk-packed
on the host so every DMA is 128 partitions x 8KB contiguous runs.

Per-core algorithm (mode "bf16x1", default):
  Stage 1 (TensorE): Y[k, j] = sum_d x[j, d] * w[k, d] as matmuls with d on
    the contraction partitions: lhsT = w^T [128, 16] per 128-d chunk,
    rhs = x^T [128, 512 positions], accumulating 4 d-chunks in PSUM.
    x and w are bf16 (input rounding dominates the error budget at ~2e-3
    against the 2e-2 gate); PSUM accumulation is fp32.
  Shift (DMA): out[i] needs sum_k Y[k, i+k] - a diagonal no compute engine
    can address. Y (cast to fp16 by DVE) is written to a DRAM scratch with
    row pitch S+K and read back with row pitch S+K+1, which lands Y[k, j+k]
    at [k, j]; row tails past S are pre-zeroed. The read lands the whole
    batch as af[128, 512] with partition p = k*8 + t covering positions
    [512t, 512t+512).
  Stage 2 (TensorE): ONE matmul per batch with a block-pattern ones
    stationary [128, 8] (ones8[k*8+t, t] = 1) sums the 16 k-rows of each
    of the 8 subtiles; ScalarE adds bias during PSUM evacuation.

Perf notes (from trace analysis of the bf16x3 predecessor at ~139-145us):
  - DMA is the roofline: ~410-420 GB/s/core sustained across the 16 HW
    engines regardless of how many DGE rings carry the traffic. Dropping
    the x-lo stream halves input bytes to 16.8 MB/core (~41 us).
  - PE streams bf16 moving operands at ~2.4 GHz, 1 col/cycle, and
    LDWEIGHTS hides under the previous matmul; stage 1 is 128 matmuls of
    512 cols (~28 us) - under the DMA envelope.
  - x chunks alternate between the Sync and Scalar HWDGE rings; scratch
    bounce / consts / output go through SWDGE (gpsimd) so they never
    head-of-line block an x prefetch.
  - Each batch's stage-2 is deferred into the next batch's stage-1 window
    (the DRAM bounce round trip is ~2-4 us); only the last batch's chain
    is exposed as tail.

Precision: x bf16 + w bf16 + fp16 Y bounce simulates to 2.1e-3 max-rel
against an fp64 reference (gate is 2e-2). Modes "bf16x3" (x split into
bf16 hi+lo pairs, ~6e-6), "f32r", "f32" kept for reference.

_split_multiwaits works around this container's walrus build accepting
only one sync-wait command per instruction.
"""

import sys

import numpy as np

if "/opt/trn_rl_repo" not in sys.path:
    sys.path.append("/opt/trn_rl_repo")

import ml_dtypes

import concourse.bass as bass
import concourse.mybir as mybir
from concourse.bass_utils import run_bass_kernel_spmd
from concourse.tile import TileContext

# Problem constants (hardcoded per the harness contract).
B, S, D, K = 32, 4096, 512, 16
N_CORES = 8
BC = B // N_CORES  # batches per core
P = 128  # SBUF partitions / contraction size
DC = D // P  # d-chunks per position
TN = 512  # positions per matmul (PSUM bank = 512 fp32)
NT = S // TN  # 512-tiles per batch

_F32 = mybir.dt.float32
_BF16 = mybir.dt.bfloat16
_F16 = mybir.dt.float16
BF = ml_dtypes.bfloat16

DEFAULT_MODE = "bf16x1"

# bf16x1 layout constants
CW = 1024  # positions per x chunk (8 KB contiguous per partition)
NCH = S // CW  # chunks per batch
W_PITCH = S + K  # scratch write row pitch
R_PITCH = S + K + 1  # scratch read row pitch (walks the k-diagonal)
SCR_SIZE = K * R_PITCH  # per-batch scratch elems


def _split_multiwaits(nc, max_waits=1):
    """This container's walrus build accepts at most one sync-wait command
    per instruction ("Too many sync wait commands" in setupSyncWait
    otherwise). Splitting a multi-wait instruction into a chain of
    same-engine single-wait carriers is semantically identical: waits are
    conjunctive and each engine executes its stream in order."""
    n = 0
    for fn in nc.m.functions:
        for blk in fn.blocks:
            out = []
            for ins in blk.instructions:
                si = getattr(ins, "sync_info", None)
                waits = list(si.on_wait) if si is not None and si.on_wait else []
                if len(waits) > max_waits:
                    extra = waits[: len(waits) - max_waits]
                    si.on_wait = waits[len(waits) - max_waits :]
                    for i in range(0, len(extra), max_waits):
                        # EVENT_SEMAPHORE is a pure wait carrier (~20-50 ns);
                        # a Drain here would flush the engine pipeline (on
                        # TensorE that costs microseconds per occurrence).
                        d = mybir.InstEventSemaphore(
                            name=nc.get_next_instruction_name(),
                            engine=ins.engine,
                            ins=[],
                            outs=[],
                            sync_info=mybir.SyncInfo(
                                on_wait=extra[i : i + max_waits], on_update=[]
                            ),
                        )
                        out.append(d)
                        n += 1
                out.append(ins)
            if len(out) != len(blk.instructions):
                blk.instructions = out
    return n


def build_nc_bf16x1(xbufs=6, pending_t=5):
    """Single-stream bf16 variant (see module docstring).

    DMA routing: x chunks alternate the Sync/Scalar HWDGE rings. The
    bounce write goes on Sync, the diagonal read on Scalar, and the
    output writes on Sync - all HWDGE, because SWDGE (gpsimd) completion
    semaphores land ~2.5-3us late, which both serializes the tail chain
    and stalls the PE mid-stream on deferred stage-2. Only consts and
    scratch-tail zeroing stay on SWDGE. To keep a waiting bounce trigger
    from head-of-line blocking the next x prefetch on its ring, batch
    b's bounce triggers are emitted after batch b+1's second chunk
    trigger (by then their waits are long satisfied)."""
    nc = bass.Bass("TRN2", debug=False)
    # x: chunk-major flat stream [b, chunk, p, dc, cw]
    xin = nc.dram_tensor("x", [BC * NCH * P * DC * CW], _BF16, kind="ExternalInput")
    wd = nc.dram_tensor("w", [P, DC * K], _BF16, kind="ExternalInput")
    ones_d = nc.dram_tensor("ones8", [P, NT], _BF16, kind="ExternalInput")
    bias_d = nc.dram_tensor("bias8", [NT, TN], _F32, kind="ExternalInput")
    zer_d = nc.dram_tensor("zer", [K, K], _BF16, kind="ExternalInput")
    out = nc.dram_tensor("out", [BC, S], _F32, kind="ExternalOutput")

    with TileContext(nc) as tc:
        with (
            tc.tile_pool(name="consts", bufs=1) as cpool,
            tc.tile_pool(name="xp", bufs=xbufs) as xpool,
            tc.tile_pool(name="yp", bufs=2) as ypool,
            tc.tile_pool(name="afp", bufs=2) as apool,
            tc.tile_pool(name="obp", bufs=2) as opool,
            tc.tile_pool(name="psy", bufs=4, space="PSUM") as psy,
            tc.tile_pool(name="pso", bufs=2, space="PSUM") as pso,
            tc.tile_pool(name="psw", bufs=2, space="PSUM") as psw,
            tc.tile_pool(name="dscr", bufs=1, space="DRAM") as dpool,
        ):
            # First instruction on the Sync ring: the first x chunk.
            # (Emitted inside the batch loop below; consts go on SWDGE so
            # they never delay it.)
            # PE warm-up: the PE drops to ~half clock after a few us of
            # idle and ramps back over ~2.5-3us. The NEFF preamble plus
            # the first chunk's DMA leave it idle for ~6us, so the first
            # real matmuls would run at half rate. Chew on garbage SBUF
            # (never written - results are discarded) from the end of
            # the preamble until the first chunk lands.
            wstat = cpool.tile([P, K], _BF16)
            wmov = cpool.tile([P, TN], _BF16)
            nc.vector.random(wstat[:, :])
            nc.vector.random(wmov[:, :])
            warm_tiles = []

            def warm(n):
                # Alternate PSUM banks so the warm matmuls pipeline
                # (same-bank WAW would serialize them at ~630ns each and
                # delay the first real matmul behind the queue).
                for i in range(n):
                    wps = psw.tile([K, TN], _F32, name="wps")
                    nc.tensor.matmul(
                        wps[:, :], wstat[:, :], wmov[:, :], start=True, stop=True
                    )
                    warm_tiles.append(wps)

            warm(8)

            wsb = cpool.tile([P, DC * K], _BF16)
            nc.gpsimd.dma_start(out=wsb[:, :], in_=wd[:, :])
            ones8 = cpool.tile([P, NT], _BF16)
            nc.gpsimd.dma_start(out=ones8[:, :], in_=ones_d[:, :])
            bsb = cpool.tile([NT, TN], _F32)
            nc.gpsimd.dma_start(out=bsb[:, :], in_=bias_d[:, :])
            zer = cpool.tile([K, K], _BF16)
            nc.gpsimd.dma_start(out=zer[:, :], in_=zer_d[:, :])

            scr = [dpool.tile([SCR_SIZE], _BF16, name=f"scr{b}") for b in range(BC)]

            def wview(b):
                return scr[b][0 : K * W_PITCH].rearrange("(k i) -> k i", i=W_PITCH)

            # Zero the row tails (reads past S walk into them).
            for b in range(BC):
                nc.gpsimd.dma_start(out=wview(b)[:, S:W_PITCH], in_=zer[:, :])

            def make_wa(b, ybuf):
                # Bounce write piece A: columns [0, 3584) only depend on
                # CASTs 0-6, so this fires ~1.4us before the batch ends
                # and its DMA hop overlaps the last tile.
                def emit():
                    nc.sync.dma_start(
                        out=wview(b)[:, 0 : 7 * TN], in_=ybuf[:, 0 : 7 * TN]
                    )

                return emit

            def make_wb(b, ybuf):
                # Bounce write piece B: the last tile's columns (small,
                # so the CAST7 -> write -> read critical path is short).
                def emit():
                    nc.sync.dma_start(
                        out=wview(b)[:, 7 * TN : S], in_=ybuf[:, 7 * TN : S]
                    )

                return emit

            def make_r(b):
                # Diagonal read (Scalar ring).
                def emit():
                    af = apool.tile([P, TN], _BF16, name="af")
                    vr = scr[b][:].rearrange("(k m) -> k m", m=R_PITCH)
                    # NOTE: the dest must be plain 2-D; the leading src
                    # dims (k, t) map onto the 128 partitions as
                    # p = k*8 + t. A partition-split rearrange on the
                    # SBUF side mis-lowers (bass merges the split back
                    # into the free axis).
                    nc.scalar.dma_start(
                        out=af[:, :],
                        in_=vr[:, 0:S].rearrange("k (t j) -> k t j", j=TN),
                    )
                    return af

                return emit

            afbox = {}

            def make_pending(b):
                # Bias-add and output write live on the otherwise-idle
                # gpsimd engine so this long-latency chain never blocks
                # an x prefetch trigger (scalar/sync engine streams are
                # in-order; a waiting instruction would head-of-line
                # block the triggers emitted after it).
                def emit(out_eng=nc.gpsimd):
                    af = afbox.pop(b)
                    po = pso.tile([NT, TN], _F32, name="po")
                    nc.tensor.matmul(
                        po[:, :], ones8[:, :], af[:, :], start=True, stop=True
                    )
                    ob = opool.tile([NT, TN], _F32, name="ob")
                    nc.vector.tensor_tensor(
                        ob[:, :], po[:, :], bsb[:, :], mybir.AluOpType.add
                    )
                    out_eng.dma_start(
                        out=out[b].rearrange("(p j) -> p j", j=TN),
                        in_=ob[:, :],
                    )

                return emit

            wa = None  # deferred bounce-write piece A of the previous batch
            wb = None  # deferred bounce-write piece B of the previous batch
            rb = None  # deferred diagonal-read trigger of the previous batch
            pending = None  # deferred stage-2 of the previous batch
            gchunk = 0
            for b in range(BC):
                ybuf = ypool.tile([K, S], _BF16)
                for ci in range(NCH):
                    xb = xpool.tile([P, DC * CW], _BF16, name="xb")
                    eng = nc.sync if gchunk % 2 == 0 else nc.scalar
                    off = gchunk * P * DC * CW
                    eng.dma_start(
                        out=xb[:, :],
                        in_=xin[off : off + P * DC * CW].rearrange(
                            "(p m) -> p m", m=DC * CW
                        ),
                    )
                    gchunk += 1
                    # The bounce triggers ride the same rings as x; each
                    # is emitted right AFTER a later x trigger so its
                    # semaphore wait never head-of-line blocks a prefetch,
                    # but as early as possible so the af round trip
                    # (~2 hops x ~2.5us) completes before the PE reaches
                    # the deferred stage-2 at pending_t.
                    if ci == 1 and wa is not None:
                        wa()
                        wb()
                        afbox[b - 1] = rb()
                        wa = wb = rb = None
                    for tt in range(CW // TN):
                        t = ci * (CW // TN) + tt
                        if t == pending_t and pending is not None:
                            pending()
                            pending = None
                        py = psy.tile([K, TN], _F32, name="py")
                        for dc in range(DC):
                            nc.tensor.matmul(
                                py[:, :],
                                wsb[:, dc * K : (dc + 1) * K],
                                xb[:, dc * CW + tt * TN : dc * CW + (tt + 1) * TN],
                                start=(dc == 0),
                                stop=(dc == DC - 1),
                            )
                        nc.vector.tensor_copy(
                            ybuf[:, t * TN : (t + 1) * TN], py[:, :]
                        )

                wa = make_wa(b, ybuf)
                wb = make_wb(b, ybuf)
                rb = make_r(b)
                pending = make_pending(b)
            # Last batch: nothing left to hide behind; emit immediately
            # (the rings are empty by now). Output goes on the Sync ring
            # so its completion - which gates the final drain - isn't
            # paying the SWDGE latency.
            warm(3)
            wa()
            wb()
            afbox[BC - 1] = rb()
            pending(out_eng=nc.sync)

            # Readers for the warm-matmul PSUM banks (the BIR verifier
            # rejects written-never-read PSUM); values never escape.
            for wps in warm_tiles[-2:]:
                nc.vector.tensor_copy(wmov[0:K, :], wps[:, :])

    _split_multiwaits(nc)
    return nc


def build_nc_bf16x3(xh_=2048, xbufs=4):
    """3-pass bf16 split-precision variant (predecessor, ~139-145 us).

    x and w are split on the host into bf16 hi+lo pairs (same total bytes
    as fp32); stage 1 computes xh*wh + xh*wl + xl*wh with fp32 PSUM
    accumulation. See git history / module docstring for details."""
    xh = xh_
    ntile = S // TN
    PITCH = S + K
    YFLAT = K * (PITCH + 1)
    _ = YFLAT

    nc = bass.Bass("TRN2", debug=False)
    xth = nc.dram_tensor("xth", [BC, D, S], _BF16, kind="ExternalInput")
    xtl = nc.dram_tensor("xtl", [BC, D, S], _BF16, kind="ExternalInput")
    wd = nc.dram_tensor("w", [P, DC * 3 * K], _BF16, kind="ExternalInput")
    bias = nc.dram_tensor("bias", [1, 1], _F32, kind="ExternalInput")
    ones_d = nc.dram_tensor("ones", [3 * K, 1], _BF16, kind="ExternalInput")
    zer_d = nc.dram_tensor("zer", [3 * K, K], _BF16, kind="ExternalInput")
    out = nc.dram_tensor("out", [BC, S], _F32, kind="ExternalOutput")

    RANGES = [(0, 3 * TN), (3 * TN, S)]
    READY = [3, ntile - 1]
    G = 3  # streams

    with TileContext(nc) as tc:
        with (
            tc.tile_pool(name="consts", bufs=1) as cpool,
            tc.tile_pool(name="xph", bufs=xbufs) as xpool_h,
            tc.tile_pool(name="xpl", bufs=xbufs) as xpool_l,
            tc.tile_pool(name="ypool", bufs=2) as ypool,
            tc.tile_pool(name="afp", bufs=4) as apool,
            tc.tile_pool(name="obp", bufs=2) as opool,
            tc.tile_pool(name="psy", bufs=4, space="PSUM") as psy,
            tc.tile_pool(name="pso", bufs=3, space="PSUM") as pso,
            tc.tile_pool(name="dscr", bufs=1, space="DRAM") as dpool,
        ):
            wsb = cpool.tile([P, DC * 3 * K], _BF16)
            nc.gpsimd.dma_start(out=wsb[:, :], in_=wd[:, :])
            bsb = cpool.tile([1, 1], _F32)
            nc.gpsimd.dma_start(out=bsb[:, :], in_=bias[:, :])
            ones = cpool.tile([3 * K, 1], _BF16)
            nc.gpsimd.dma_start(out=ones[:, :], in_=ones_d[:, :])
            zer = cpool.tile([3 * K, K], _BF16)
            nc.gpsimd.dma_start(out=zer[:, :], in_=zer_d[:, :])

            scr = {}
            for b in range(BC):
                for r, (lo, hi) in enumerate(RANGES):
                    w_ = hi - lo + K
                    scr[(b, r)] = dpool.tile(
                        [G * K * (w_ + 1)], _BF16, name=f"scr{r}_{b}"
                    )

            for b in range(BC):
                lo, hi = RANGES[-1]
                w_ = hi - lo + K
                s = scr[(b, len(RANGES) - 1)]
                v = s[:].rearrange("(g kr) -> g kr", g=G)[
                    :, 0 : K * w_
                ].rearrange("g (k r) -> g k r", r=w_)
                nc.gpsimd.dma_start(out=v[:, :, w_ - K : w_], in_=zer[:, :])

            def bounce(b, r, yb):
                lo, hi = RANGES[r]
                w_ = hi - lo + K
                wend = hi + K if r < len(RANGES) - 1 else S
                af = apool.tile([G * K, hi - lo], _BF16, name="af")
                s = scr[(b, r)]
                blk = s[:].rearrange("(g kr) -> g kr", g=G)
                rows = blk[:, 0 : K * w_].rearrange("g (k r) -> g k r", r=w_)
                for gi, yrow in enumerate((0, 32, 64)):
                    nc.gpsimd.dma_start(
                        out=rows[gi, :, 0 : wend - lo],
                        in_=yb[yrow : yrow + K, lo:wend],
                    )
                diag = blk[:, 0 : K * (w_ + 1)].rearrange(
                    "g (k r) -> g k r", r=w_ + 1
                )
                nc.gpsimd.dma_start(
                    out=af[:, :], in_=diag[:, :, 0 : hi - lo]
                )
                return af

            def stage2(ob, af, r):
                lo, hi = RANGES[r]
                for t2 in range(lo // TN, hi // TN):
                    po = pso.tile([1, TN], _F32, name="po")
                    j = t2 * TN - lo
                    nc.tensor.matmul(
                        po[:, :],
                        ones[:, :],
                        af[:, j : j + TN],
                        start=True,
                        stop=True,
                    )
                    nc.scalar.add(
                        ob[:, t2 * TN : (t2 + 1) * TN], po[:, :], bsb[0:1, 0:1]
                    )

            pending = None
            for b in range(BC):
                yb = ypool.tile([3 * 32, S], _BF16)
                ob = opool.tile([1, S], _F32)
                afs = {}
                if b == 0:
                    chunks = [(0, TN), (TN, TN)]
                    if xh > 2 * TN:
                        chunks.append((2 * TN, xh - 2 * TN))
                    chunks += [(i, xh) for i in range(xh, S, xh)]
                else:
                    chunks = [(i, xh) for i in range(0, S, xh)]
                for c0, cw in chunks:
                    xbh = xpool_h.tile([P, DC * xh], _BF16, name="xbh")
                    nc.sync.dma_start(
                        out=xbh[:, 0 : DC * cw].rearrange(
                            "p (dc n) -> p dc n", n=cw
                        ),
                        in_=xth[b][:, c0 : c0 + cw].rearrange(
                            "(dc p) n -> p dc n", p=P
                        ),
                    )
                    xbl = xpool_l.tile([P, DC * xh], _BF16, name="xbl")
                    nc.scalar.dma_start(
                        out=xbl[:, 0 : DC * cw].rearrange(
                            "p (dc n) -> p dc n", n=cw
                        ),
                        in_=xtl[b][:, c0 : c0 + cw].rearrange(
                            "(dc p) n -> p dc n", p=P
                        ),
                    )
                    for tt in range(cw // TN):
                        t = (c0 + tt * TN) // TN
                        if t == 2 and pending is not None:
                            pending()
                            pending = None
                        py48 = psy.tile([3 * K, TN], _F32, name="py48")
                        for dc in range(DC):
                            xsl = slice(
                                dc * cw + tt * TN, dc * cw + (tt + 1) * TN
                            )
                            nc.tensor.matmul(
                                py48[:, :],
                                wsb[:, dc * 3 * K : (dc + 1) * 3 * K],
                                xbh[:, xsl],
                                start=(dc == 0),
                                stop=False,
                            )
                        for dc in range(DC):
                            xsl = slice(
                                dc * cw + tt * TN, dc * cw + (tt + 1) * TN
                            )
                            nc.tensor.matmul(
                                py48[2 * K : 3 * K, :],
                                wsb[:, dc * 3 * K : dc * 3 * K + K],
                                xbl[:, xsl],
                                start=False,
                                stop=(dc == DC - 1),
                            )
                        yhs = yb[0:K, t * TN : (t + 1) * TN]
                        nc.vector.tensor_copy(yhs, py48[0:K, :])
                        nc.vector.tensor_tensor(
                            yb[32 : 32 + K, t * TN : (t + 1) * TN],
                            py48[0:K, :],
                            yhs,
                            mybir.AluOpType.subtract,
                        )
                        nc.vector.tensor_copy(
                            yb[64 : 64 + K, t * TN : (t + 1) * TN],
                            py48[2 * K : 3 * K, :],
                        )
                        for r, rdy in enumerate(READY):
                            if t == rdy:
                                afs[r] = bounce(b, r, yb)
                        if t == 5:
                            stage2(ob, afs[0], 0)

                def make_pending(b=b, afs=afs, ob=ob):
                    def emit():
                        stage2(ob, afs[1], 1)
                        nc.gpsimd.dma_start(out=out[b : b + 1, :], in_=ob[:, :])

                    return emit

                pending = make_pending()
            if pending is not None:
                pending()

    _split_multiwaits(nc)
    return nc


def build_nc_simple(mm_dt):
    """Single-pass variant: one x tensor / one w tensor of dtype mm_dt."""
    XH = 2048
    NH = S // XH
    NTH = XH // TN
    PITCH = S + K
    DIAG = PITCH + 1
    YFLAT = K * DIAG

    nc = bass.Bass("TRN2", debug=False)
    xt = nc.dram_tensor("xt", [BC, D, S], mm_dt, kind="ExternalInput")
    w = nc.dram_tensor("w", [P, DC * K], mm_dt, kind="ExternalInput")
    bias = nc.dram_tensor("bias", [1, 1], _F32, kind="ExternalInput")
    ones_d = nc.dram_tensor("ones", [K, 1], mm_dt, kind="ExternalInput")
    zer_d = nc.dram_tensor("zer", [K, K], mm_dt, kind="ExternalInput")
    out = nc.dram_tensor("out", [BC, S], _F32, kind="ExternalOutput")

    with TileContext(nc) as tc:
        with (
            tc.tile_pool(name="consts", bufs=1) as cpool,
            tc.tile_pool(name="xp", bufs=2) as xpool,
            tc.tile_pool(name="yp", bufs=2) as ypool,
            tc.tile_pool(name="afp", bufs=2) as apool,
            tc.tile_pool(name="obp", bufs=2) as opool,
            tc.tile_pool(name="psy", bufs=2, space="PSUM") as psy,
            tc.tile_pool(name="pso", bufs=2, space="PSUM") as pso,
            tc.tile_pool(name="dscr", bufs=1, space="DRAM") as dpool,
        ):
            wsb = cpool.tile([P, DC * K], mm_dt)
            nc.sync.dma_start(out=wsb[:, :], in_=w[:, :])
            bsb = cpool.tile([1, 1], _F32)
            nc.sync.dma_start(out=bsb[:, :], in_=bias[:, :])
            ones = cpool.tile([K, 1], mm_dt)
            nc.sync.dma_start(out=ones[:, :], in_=ones_d[:, :])
            zer = cpool.tile([K, K], mm_dt)
            nc.sync.dma_start(out=zer[:, :], in_=zer_d[:, :])
            yscr = dpool.tile([BC, YFLAT], mm_dt)

            for b in range(BC):
                tail = yscr[b, 0 : K * PITCH].rearrange("(k r) -> k r", r=PITCH)[
                    :, S:PITCH
                ]
                nc.sync.dma_start(out=tail, in_=zer[:, :])

            for b in range(BC):
                ybuf = ypool.tile([K, S], mm_dt)
                for h in range(NH):
                    xb = xpool.tile([P, DC * XH], mm_dt)
                    nc.sync.dma_start(
                        out=xb[:, :].rearrange("p (dc n) -> p dc n", n=XH),
                        in_=xt[b][:, h * XH : (h + 1) * XH].rearrange(
                            "(dc p) n -> p dc n", p=P
                        ),
                    )
                    for tt in range(NTH):
                        t = h * NTH + tt
                        py = psy.tile([K, TN], _F32)
                        for dc in range(DC):
                            nc.tensor.matmul(
                                py[:, :],
                                wsb[:, dc * K : (dc + 1) * K],
                                xb[:, dc * XH + tt * TN : dc * XH + (tt + 1) * TN],
                                start=(dc == 0),
                                stop=(dc == DC - 1),
                            )
                        nc.vector.tensor_copy(
                            ybuf[:, t * TN : (t + 1) * TN], py[:, :]
                        )

                ywr = yscr[b, 0 : K * PITCH].rearrange("(k r) -> k r", r=PITCH)[
                    :, 0:S
                ]
                nc.sync.dma_start(out=ywr, in_=ybuf[:, :])

                af = apool.tile([K, S], mm_dt)
                ard = yscr[b, :].rearrange("(k r) -> k r", r=DIAG)[:, 0:S]
                nc.sync.dma_start(out=af, in_=ard)

                ob = opool.tile([1, S], _F32)
                for t in range(S // TN):
                    po = pso.tile([1, TN], _F32)
                    nc.tensor.matmul(
                        po[:, :],
                        ones[:, :],
                        af[:, t * TN : (t + 1) * TN],
                        start=True,
                        stop=True,
                    )
                    nc.scalar.add(
                        ob[:, t * TN : (t + 1) * TN], po[:, :], bsb[0:1, 0:1]
                    )
                nc.sync.dma_start(out=out[b : b + 1, :], in_=ob[:, :])

    _split_multiwaits(nc)
    return nc


_NC_CACHE = {}


def _get_nc(mode):
    if mode not in _NC_CACHE:
        if mode == "bf16x1":
            _NC_CACHE[mode] = build_nc_bf16x1()
        elif mode == "bf16x3":
            _NC_CACHE[mode] = build_nc_bf16x3()
        elif mode == "f32r":
            _NC_CACHE[mode] = build_nc_simple(mybir.dt.float32r)
        elif mode == "f32":
            _NC_CACHE[mode] = build_nc_simple(mybir.dt.float32)
        else:
            raise ValueError(mode)
    return _NC_CACHE[mode]


def _wl_layout(f):
    # [p, dc*K + k] = w[k, dc*128 + p]
    return np.ascontiguousarray(
        f.reshape(K, DC, P).transpose(2, 1, 0).reshape(P, DC * K)
    )


def _prep_in_maps(embedded, filt, bias, mode):
    embedded = np.ascontiguousarray(embedded, dtype=np.float32)
    filt = np.ascontiguousarray(filt, dtype=np.float32)
    bias = np.ascontiguousarray(bias, dtype=np.float32)

    in_maps = []
    if mode == "bf16x1":
        wcat = _wl_layout(filt).astype(BF)
        ones8 = np.zeros((P, NT), dtype=BF)
        for k in range(K):
            for t in range(NT):
                ones8[k * NT + t, t] = 1.0
        bias8 = np.full((NT, TN), bias[0], dtype=np.float32)
        zer16 = np.zeros((K, K), dtype=BF)
        xh = embedded.astype(BF)
        for c in range(N_CORES):
            xc = xh[c * BC : (c + 1) * BC]  # [BC, S, D]
            xt = xc.transpose(0, 2, 1)  # [BC, D, S]
            xr = xt.reshape(BC, DC, P, NCH, CW)
            xp = np.ascontiguousarray(xr.transpose(0, 3, 2, 1, 4))
            in_maps.append(
                {
                    "x": xp.reshape(-1),
                    "w": wcat,
                    "ones8": ones8,
                    "bias8": bias8,
                    "zer": zer16,
                }
            )
    elif mode == "bf16x3":
        b11 = bias.reshape(1, 1)
        wh = filt.astype(BF)
        wlo = (filt - wh.astype(np.float32)).astype(BF)
        whl = _wl_layout(wh.astype(np.float32)).reshape(P, DC, K)
        wll = _wl_layout(wlo.astype(np.float32)).reshape(P, DC, K)
        wcat = np.zeros((P, DC, 3 * K), dtype=np.float32)
        wcat[:, :, 0:K] = whl
        wcat[:, :, 2 * K : 3 * K] = wll
        wcat = wcat.reshape(P, DC * 3 * K).astype(BF)
        ones16 = np.ones((3 * K, 1), dtype=BF)
        zer16 = np.zeros((3 * K, K), dtype=BF)
        xh = embedded.astype(BF)
        xl = (embedded - xh.astype(np.float32)).astype(BF)
        for c in range(N_CORES):
            sl = slice(c * BC, (c + 1) * BC)
            xthc = np.ascontiguousarray(xh[sl].transpose(0, 2, 1))
            xtlc = np.ascontiguousarray(xl[sl].transpose(0, 2, 1))
            in_maps.append(
                {
                    "xth": xthc,
                    "xtl": xtlc,
                    "w": wcat,
                    "bias": b11,
                    "ones": ones16,
                    "zer": zer16,
                }
            )
    else:
        b11 = bias.reshape(1, 1)
        wl = _wl_layout(filt)
        ones16 = np.ones((K, 1), dtype=np.float32)
        zer16 = np.zeros((K, K), dtype=np.float32)
        for c in range(N_CORES):
            xc = embedded[c * BC : (c + 1) * BC]
            xtc = np.ascontiguousarray(xc.transpose(0, 2, 1))
            in_maps.append(
                {"xt": xtc, "w": wl, "bias": b11, "ones": ones16, "zer": zer16}
            )
    return in_maps


def run(embedded, filt, bias, mode=DEFAULT_MODE, trace=False, **spmd_kwargs):
    nc = _get_nc(mode)
    in_maps = _prep_in_maps(embedded, filt, bias, mode)
    res = run_bass_kernel_spmd(
        nc, in_maps, list(range(N_CORES)), trace=trace, **spmd_kwargs
    )
    out = np.concatenate([res.results[c]["out"] for c in range(N_CORES)], axis=0)
    return out.astype(np.float32), res


def kernel(embedded, filt, bias):
    out, _ = run(embedded, filt, bias)
    return out
